# revision 8
# baseline (speedup 1.0000x reference)
"""MeanPoolAggregator Trainium2 kernel.

Computes out = mean_k(features[neigh_idx], axis=1) @ W.T  for
neigh_idx [50000, 16] int, features [100000, 256] f32, W [128, 256] f32.

Strategy: data-parallel over the 50000 batch rows across 8 NeuronCores.
Each core processes 6272 (padded) rows in 49 tiles of 128. Per tile one
indirect (gather) DMA pulls the 128x16 neighbor feature rows (2 MB) into
SBUF, a DVE binary tree reduces over the 16 neighbors, TensorE transposes
the [128, 256] accumulator and contracts it with W^T/16 to produce the
[128 rows, 128 pool] output tile. Memory-bound on the gather
(~100 MB/core of 1 KB random reads).
"""

from contextlib import ExitStack

import numpy as np

import concourse.bass as bass
import concourse.bacc as bacc
import concourse.mybir as mybir
import concourse.tile as tile
from concourse.bass_utils import run_bass_kernel_spmd
from concourse.masks import make_identity

N_BATCH = 50000
N_UNIQUE = 100000
K = 16
HID = 256
POOL = 128

N_CORES = 8
P = 128
TILES_PER_CORE = 49  # ceil(50000 / 8 / 128)
ROWS_PER_CORE = TILES_PER_CORE * P  # 6272
N_PAD = ROWS_PER_CORE * N_CORES  # 50176

F32 = mybir.dt.float32


def _emit(tc: tile.TileContext, out, idx, feats, wt):
    nc = tc.nc
    with ExitStack() as ctx:
        const_pool = ctx.enter_context(tc.tile_pool(name="const", bufs=1))
        idx_pool = ctx.enter_context(tc.tile_pool(name="idx", bufs=4))
        g_pool = ctx.enter_context(tc.tile_pool(name="g", bufs=3))
        red_pool = ctx.enter_context(tc.tile_pool(name="red", bufs=2))
        acc_pool = ctx.enter_context(tc.tile_pool(name="acc", bufs=2))
        accT_pool = ctx.enter_context(tc.tile_pool(name="accT", bufs=2))
        out_pool = ctx.enter_context(tc.tile_pool(name="outsb", bufs=2))
        psum_pool = ctx.enter_context(tc.tile_pool(name="psum", bufs=2, space="PSUM"))

        ident = const_pool.tile([P, P], F32)
        make_identity(nc, ident[:])

        # WT = W.T [256, 128] lives as two [128, 128] chunks side by side.
        wt_sb = const_pool.tile([P, 2 * POOL], F32)
        nc.sync.dma_start(wt_sb[:, 0:POOL], wt[0:P, :])
        nc.sync.dma_start(wt_sb[:, POOL : 2 * POOL], wt[P : 2 * P, :])

        for t in range(TILES_PER_CORE):
            idx_sb = idx_pool.tile([P, K], mybir.dt.int32)
            nc.sync.dma_start(idx_sb[:], idx[t * P : (t + 1) * P, :])

            # HW indirect DMA consumes one offset per dest partition-row, so
            # gather each neighbor slot k separately: g_k[p, :] =
            # features[idx[p, k], :]. Separate tiles per k keep Tile from
            # serializing the transfers on same-tile WAW completions.
            gs = []
            for k in range(K):
                g_k = g_pool.tile([P, HID], F32, tag=f"g{k}")
                nc.gpsimd.indirect_dma_start(
                    out=g_k[:],
                    out_offset=None,
                    in_=feats[:],
                    in_offset=bass.IndirectOffsetOnAxis(
                        ap=idx_sb[:, k : k + 1], axis=0
                    ),
                )
                gs.append(g_k)

            # Pairwise binary-tree sum over the 16 gathered tiles.
            h1 = red_pool.tile([P, 8 * HID], F32, tag="h1")
            for j in range(8):
                nc.vector.tensor_add(
                    h1[:, j * HID : (j + 1) * HID], gs[2 * j][:], gs[2 * j + 1][:]
                )
            h2 = red_pool.tile([P, 4 * HID], F32, tag="h2")
            nc.vector.tensor_add(h2[:], h1[:, 0 : 4 * HID], h1[:, 4 * HID : 8 * HID])
            h3 = red_pool.tile([P, 2 * HID], F32, tag="h3")
            nc.vector.tensor_add(h3[:], h2[:, 0 : 2 * HID], h2[:, 2 * HID : 4 * HID])
            acc = acc_pool.tile([P, HID], F32)
            nc.vector.tensor_add(acc[:], h3[:, 0:HID], h3[:, HID : 2 * HID])

            # accT[h, n] = acc[n, h], done in two 128x128 blocks via PE.
            accT = accT_pool.tile([P, 2 * P], F32)
            for c in range(2):
                accT_ps = psum_pool.tile([P, P], F32, tag=f"accT{c}")
                nc.tensor.transpose(
                    accT_ps[:], acc[:, c * P : (c + 1) * P], ident[:]
                )
                # PSUM -> SBUF copy with the 1/K mean folded in.
                nc.vector.tensor_scalar_mul(
                    accT[:, c * P : (c + 1) * P], accT_ps[:], 1.0 / K
                )

            # out[n, p] = sum_h accT[h, n] * wt[h, p]   (accT pre-scaled by 1/K)
            out_ps = psum_pool.tile([P, POOL], F32, tag="out")
            for c in range(2):
                nc.tensor.matmul(
                    out_ps[:],
                    lhsT=accT[:, c * P : (c + 1) * P],
                    rhs=wt_sb[:, c * POOL : (c + 1) * POOL],
                    start=(c == 0),
                    stop=(c == 1),
                )
            out_sb = out_pool.tile([P, POOL], F32)
            nc.vector.tensor_copy(out_sb[:], out_ps[:])
            nc.sync.dma_start(out[t * P : (t + 1) * P, :], out_sb[:])


def build_program():
    nc = bacc.Bacc(
        "TRN2",
        target_bir_lowering=False,
        debug=False,
        enable_asserts=False,
        num_devices=N_CORES,
    )
    idx_d = nc.dram_tensor(
        "neigh_idx", [ROWS_PER_CORE, K], mybir.dt.int32, kind="ExternalInput"
    )
    feat_d = nc.dram_tensor("features", [N_UNIQUE, HID], F32, kind="ExternalInput")
    wt_d = nc.dram_tensor("wt", [HID, POOL], F32, kind="ExternalInput")
    out_d = nc.dram_tensor("out", [ROWS_PER_CORE, POOL], F32, kind="ExternalOutput")
    with tile.TileContext(nc) as tc:
        _emit(tc, out_d.ap(), idx_d.ap(), feat_d.ap(), wt_d.ap())
    nc.compile()
    return nc


def make_in_maps(neigh_idx, features, W):
    neigh_idx = np.asarray(neigh_idx).astype(np.int32)
    features = np.ascontiguousarray(np.asarray(features, dtype=np.float32))
    W = np.asarray(W, dtype=np.float32)
    wt = np.ascontiguousarray(W.T)  # [HID, POOL]

    idx_pad = np.zeros((N_PAD, K), np.int32)
    idx_pad[:N_BATCH] = neigh_idx
    shards = idx_pad.reshape(N_CORES, ROWS_PER_CORE, K)
    return [
        {
            "neigh_idx": np.ascontiguousarray(shards[c]),
            "features": features,
            "wt": wt,
        }
        for c in range(N_CORES)
    ]


def kernel(neigh_idx, features, W, **run_kwargs):
    nc = build_program()
    in_maps = make_in_maps(neigh_idx, features, W)
    res = run_bass_kernel_spmd(
        nc, in_maps, core_ids=list(range(N_CORES)), **run_kwargs
    )
    out = np.concatenate([res.results[c]["out"] for c in range(N_CORES)], axis=0)
    if run_kwargs:
        return out[:N_BATCH], res
    return out[:N_BATCH]
